# revision 5
# baseline (speedup 1.0000x reference)
"""Trainium2 Bass kernel for nn_CurvatureLoss: softmax over 4 classes ->
3 probability maps -> fused curvature-stencil chain -> masked-mean loss.

Strategy: pure data-parallel over batch (8 samples -> 8 cores). Per core,
the [4,1024,1024] sample is processed in 9 overlapping 128-row slabs
(stride 122; each slab owns its interior rows). Per slab:
  - one casting gpsimd DMA loads all 4 class rows as fp16
  - one ACT exp over [128,4096], DVE sums + reciprocal -> 3 prob maps
  - per map, H-direction stencils are truncated-band 128x128 matmuls on
    the tensor engine (lap, hyy, hxx, hxy via shifted-identity bands);
    gy comes from a partition-shifted DVE subtract of the fp16 lap copy
  - the nonlinear chain (quadratic forms, D^-1.5 via ln/exp on ACT) is
    spread across DVE / ACT / GpSimd with sign flipped so the final
    relu+sum and count accumulate via cheap tensor_scalar ops
Per-slab/map s (sum relu(-curv)) and c (count curv<0) land in fp32
accumulator columns; the host slices owned rows and does the tiny
masked-mean reduction.
"""
import sys

if "/opt/trn_rl_repo" not in sys.path:
    sys.path.insert(0, "/opt/trn_rl_repo")

import numpy as np

P = 128
H = W = 1024
N_CORES = 8
STARTS = [0, 122, 244, 366, 488, 610, 732, 854, 896]
OWNED = [(0, 125)] + [(3, 125)] * 7 + [(83, 128)]
NSLAB = len(STARTS)
ACC_COLS = NSLAB * 3 * 2


def _band_weights():
    """fp16 lhsT weight blocks [128, 7*128]:
    M1.T (lap), M2.T (gy), M3.T (hyy), I, -I, 2I, -2I."""
    SyP = np.eye(P, k=1, dtype=np.float64)   # (SyP x)[h] = x[h+1]
    SyM = np.eye(P, k=-1, dtype=np.float64)  # (SyM x)[h] = x[h-1]
    I = np.eye(P, dtype=np.float64)
    M1 = SyP + SyM - 4 * I                   # lap = M1 @ p + (E + W)
    M2 = SyP - SyM                           # gy = M2 @ lap
    M3 = (2 * I - SyP - SyM) @ M2            # hyy = M3 @ lap
    wts = np.concatenate(
        [M1.T, M2.T, M3.T, I, -I, 2 * I, -2 * I], axis=1).astype(np.float16)
    return np.ascontiguousarray(wts)         # [128, 896] fp16


_CACHE = {}


def _build_program():
    import concourse.bacc as bacc
    import concourse.mybir as mybir
    from concourse.tile import TileContext

    f32 = mybir.dt.float32
    f16 = mybir.dt.float16
    Alu = mybir.AluOpType
    Act = mybir.ActivationFunctionType

    nc = bacc.Bacc("TRN2", target_bir_lowering=False, debug=False,
                   enable_asserts=False, num_devices=N_CORES)
    pred = nc.dram_tensor("pred", [4, H, W], f32, kind="ExternalInput").ap()
    wts = nc.dram_tensor("wts", [P, 7 * P], f16, kind="ExternalInput").ap()
    accd = nc.dram_tensor("acc", [P, ACC_COLS], f32, kind="ExternalOutput").ap()

    with TileContext(nc) as tc:
        with tc.tile_pool(name="const", bufs=1) as cpool, \
             tc.tile_pool(name="big", bufs=2) as bpool, \
             tc.tile_pool(name="work", bufs=3) as pool, \
             tc.tile_pool(name="pA", bufs=1, space="PSUM") as ppa, \
             tc.tile_pool(name="pB", bufs=1, space="PSUM") as ppb, \
             tc.tile_pool(name="pC", bufs=1, space="PSUM") as ppc, \
             tc.tile_pool(name="pD", bufs=1, space="PSUM") as ppd, \
             nc.allow_low_precision(reason="fp16 chain validated vs reference"):
            wt = cpool.tile([P, 7 * P], f16)
            nc.sync.dma_start(out=wt[:], in_=wts)
            wM1 = wt[:, 0:P]
            wM2 = wt[:, P:2 * P]
            wM3 = wt[:, 2 * P:3 * P]
            wI = wt[:, 3 * P:4 * P]
            wmI = wt[:, 4 * P:5 * P]
            w2I = wt[:, 5 * P:6 * P]
            wm2I = wt[:, 6 * P:7 * P]
            acc = cpool.tile([P, ACC_COLS], f32)
            nc.vector.memset(acc[:], 0.0)
            hs = cpool.tile([P, 1], f32)          # 1/sqrt(2) bias for squares
            nc.vector.memset(hs[:], 0.7071067811865476)

            for si, st in enumerate(STARTS):
                # ---- load + softmax -------------------------------------
                xt = bpool.tile([P, 4, W], f16, tag="xt")
                nc.gpsimd.dma_start(
                    out=xt[:], in_=pred[:, st:st + P, :].transpose([1, 0, 2]))
                ex = bpool.tile([P, 4, W], f16, tag="ex")
                nc.scalar.activation(out=ex[:], in_=xt[:], func=Act.Exp)
                a12 = pool.tile([P, W], f16, tag="a12")
                nc.vector.tensor_add(a12, ex[:, 1, :], ex[:, 2, :])
                b03 = pool.tile([P, W], f16, tag="b03")
                nc.vector.tensor_add(b03, ex[:, 0, :], ex[:, 3, :])
                SS = pool.tile([P, W], f16, tag="SS")
                nc.vector.tensor_add(SS, a12, b03)
                rS = pool.tile([P, W], f16, tag="rS")
                nc.vector.reciprocal(rS, SS)
                probs = bpool.tile([P, 3, W + 2], f16, tag="probs")
                nc.gpsimd.memset(probs[:, :, 0:1], 0.0)
                nc.gpsimd.memset(probs[:, :, W + 1:W + 2], 0.0)
                nc.vector.tensor_mul(probs[:, 0, 1:W + 1], ex[:, 1, :], rS)
                nc.vector.tensor_mul(probs[:, 1, 1:W + 1], a12, rS)
                nc.vector.tensor_mul(probs[:, 2, 1:W + 1], ex[:, 3, :], rS)

                for m in range(3):
                    pC = probs[:, m, 1:W + 1]
                    pE = probs[:, m, 2:W + 2]
                    pW_ = probs[:, m, 0:W]
                    # ---- lap on PE, evac via ACT ------------------------
                    lap_ps = ppa.tile([P, W], f32, tag="pa")
                    for hf in range(2):
                        sl = slice(hf * 512, (hf + 1) * 512)
                        nc.tensor.matmul(lap_ps[:, sl], lhsT=wM1, rhs=pC[:, sl],
                                         start=True, stop=False)
                        nc.tensor.matmul(lap_ps[:, sl], lhsT=wI, rhs=pE[:, sl],
                                         start=False, stop=False)
                        nc.tensor.matmul(lap_ps[:, sl], lhsT=wI, rhs=pW_[:, sl],
                                         start=False, stop=True)
                    lap = pool.tile([P, W + 4], f16, tag="lap")
                    nc.gpsimd.memset(lap[:, 0:2], 0.0)
                    nc.gpsimd.memset(lap[:, W + 2:W + 4], 0.0)
                    nc.scalar.activation(out=lap[:, 2:W + 2], in_=lap_ps,
                                         func=Act.Copy)
                    lC = lap[:, 2:W + 2]
                    lE = lap[:, 3:W + 3]
                    lW_ = lap[:, 1:W + 1]
                    lE2 = lap[:, 4:W + 4]
                    lW2 = lap[:, 0:W]
                    # ---- remaining PE fields ----------------------------
                    gy_ps = ppd.tile([P, W], f32, tag="pd")
                    hyy_ps = ppc.tile([P, W], f32, tag="pc")
                    hxx_ps = ppb.tile([P, W], f32, tag="pb")
                    hxy_ps = ppa.tile([P, W], f32, tag="pa")
                    for hf in range(2):
                        sl = slice(hf * 512, (hf + 1) * 512)
                        nc.tensor.matmul(gy_ps[:, sl], lhsT=wM2, rhs=lC[:, sl],
                                         start=True, stop=True)
                        nc.tensor.matmul(hyy_ps[:, sl], lhsT=wM3, rhs=lC[:, sl],
                                         start=True, stop=True)
                        # hxx = 2*lap_E - 2*lap_W - lap_E2 + lap_W2
                        nc.tensor.matmul(hxx_ps[:, sl], lhsT=w2I, rhs=lE[:, sl],
                                         start=True, stop=False)
                        nc.tensor.matmul(hxx_ps[:, sl], lhsT=wm2I, rhs=lW_[:, sl],
                                         start=False, stop=False)
                        nc.tensor.matmul(hxx_ps[:, sl], lhsT=wmI, rhs=lE2[:, sl],
                                         start=False, stop=False)
                        nc.tensor.matmul(hxx_ps[:, sl], lhsT=wI, rhs=lW2[:, sl],
                                         start=False, stop=True)
                        # hxy = lap_E2 + lap_W2 - 2*lap
                        nc.tensor.matmul(hxy_ps[:, sl], lhsT=wI, rhs=lE2[:, sl],
                                         start=True, stop=False)
                        nc.tensor.matmul(hxy_ps[:, sl], lhsT=wI, rhs=lW2[:, sl],
                                         start=False, stop=False)
                        nc.tensor.matmul(hxy_ps[:, sl], lhsT=wm2I, rhs=lC[:, sl],
                                         start=False, stop=True)
                    # ---- first-derivative field gx on DVE ---------------
                    gx = pool.tile([P, W], f16, tag="gx")
                    nc.vector.tensor_sub(gx, lE, lW_)
                    # ---- nonlinear chain --------------------------------
                    A = pool.tile([P, W], f16, tag="A")   # 0.5*(1+gy)^2
                    nc.scalar.activation(out=A, in_=gy_ps, func=Act.Square,
                                         scale=0.7071067811865476, bias=hs[:])
                    C2 = pool.tile([P, W], f16, tag="C2")  # 0.5*(1+gx)^2
                    nc.scalar.activation(out=C2, in_=gx, func=Act.Square,
                                         scale=0.7071067811865476, bias=hs[:])
                    sq2 = pool.tile([P, W], f16, tag="sq2")
                    nc.scalar.activation(out=sq2, in_=gy_ps, func=Act.Square)
                    T1 = pool.tile([P, W], f16, tag="T1")
                    nc.vector.tensor_mul(T1, hxx_ps, A)   # hxx * 0.5(1+gy)^2
                    G = pool.tile([P, W], f16, tag="G")
                    nc.vector.tensor_mul(G, gx, gy_ps)
                    G2 = pool.tile([P, W], f16, tag="G2")
                    nc.vector.tensor_mul(G2, G, hxy_ps)   # gx*gy*hxy
                    P2 = pool.tile([P, W], f16, tag="P2")
                    nc.vector.tensor_mul(P2, hyy_ps, C2)  # hyy * 0.5(1+gx)^2
                    tn = pool.tile([P, W], f16, tag="tn")
                    nc.gpsimd.tensor_sub(tn, G2, T1)
                    numh = pool.tile([P, W], f16, tag="numh")
                    nc.gpsimd.tensor_sub(numh, tn, P2)    # = -num/2
                    sq1 = pool.tile([P, W], f16, tag="sq1")
                    nc.vector.tensor_mul(sq1, gx, gx)
                    ds = pool.tile([P, W], f16, tag="ds")
                    nc.gpsimd.tensor_add(ds, sq1, sq2)    # gx^2 + gy^2
                    lnD = pool.tile([P, W], f16, tag="lnD")
                    nc.scalar.activation(out=lnD, in_=ds, func=Act.Ln,
                                         bias=1.0)        # ln(1+gx^2+gy^2)
                    ee = pool.tile([P, W], f16, tag="ee")
                    nc.scalar.activation(out=ee, in_=lnD, func=Act.Exp,
                                         scale=-1.5)      # D^-1.5
                    t3 = pool.tile([P, W], f16, tag="t3")
                    nc.vector.tensor_mul(t3, numh, ee)    # = -curv
                    col = (si * 3 + m) * 2
                    # s: sum relu(-curv) over the row
                    nc.vector.tensor_scalar(
                        out=t3, in0=t3, scalar1=0.0, scalar2=None,
                        op0=Alu.max, op1=Alu.add,
                        accum_out=acc[:, col:col + 1])
                    # c: count curv < 0  <=>  numh > 0
                    cnt = pool.tile([P, W], f16, tag="cnt")
                    nc.vector.tensor_scalar(
                        out=cnt, in0=numh, scalar1=0.0, scalar2=None,
                        op0=Alu.is_gt, op1=Alu.add,
                        accum_out=acc[:, col + 1:col + 2])

            nc.sync.dma_start(out=accd, in_=acc[:])
    nc.compile()
    return nc


def _get_program():
    if "nc" not in _CACHE:
        _CACHE["nc"] = _build_program()
    return _CACHE["nc"]


def _in_maps(pred_np):
    wts = _band_weights()
    return [{"pred": np.ascontiguousarray(pred_np[b]), "wts": wts}
            for b in range(N_CORES)]


def _run_device(pred_np):
    from concourse import bass_utils
    nc = _get_program()
    res = bass_utils.run_bass_kernel_spmd(nc, _in_maps(pred_np),
                                          core_ids=list(range(N_CORES)))
    return [res.results[b]["acc"] for b in range(N_CORES)]


def _host_reduce(accs):
    total = 0.0
    for b in range(N_CORES):
        a = accs[b].astype(np.float64)
        for m in range(3):
            s = 0.0
            c = 0.0
            for si in range(NSLAB):
                lo, hi = OWNED[si]
                col = (si * 3 + m) * 2
                s += a[lo:hi, col].sum()
                c += a[lo:hi, col + 1].sum()
            if c > 0:
                total += s / max(c, 1.0)
    return np.float32(total)


def kernel(pred, target=None):
    assert pred.shape == (N_CORES, 4, H, W)
    accs = _run_device(np.asarray(pred, dtype=np.float32))
    return _host_reduce(accs)
